# revision 35
# baseline (speedup 1.0000x reference)
"""Trainium2 Bass kernel for FGAEmbedder (B=32, T=1024, IN=1536, D=768).

Math (identical to the reference up to float reassociation):
    h  = relu(x @ W1^T + b1)           [B,T,IN]
    u  = h @ W2^T + b2                 [B,T,D]
    e  = relu(u @ We^T + be)
    un = e @ Wr^T + br                 [B,T]
    xe = u @ Wx^T + bx ; ye = u @ Wy^T + by
    pw[t] = mean_s cos(xe[t], ye[s]) = (xe[t] . ybar) / ||xe[t]||,
            ybar = mean_s ye[s]/||ye[s]||      (the TxT matrix never exists)
    out = sum_t softmax(rw0*un + rw1*pw)[t] * u[t]

Sharding: data-parallel over batch, 4 batches per core, weights replicated,
no collectives.  Activations are feature-major on chip ([feat, tok]).

Precision strategy: fc1/fc2 run in fp16 (u directly forms the output), but
every matmul that only feeds the softmax scores runs in fp8e4 with DoubleRow
double-pumping (2 contraction rows per cycle): une, un_red, pw_x, pw_y, the
sum-of-squares row reductions, and the q = xe.ybar matmul.  Score errors are
diluted through the softmax; measured end-to-end rel err ~7e-3 vs the 2e-2
gate.  Softmax skips the max subtraction (scores are bounded ~3), which
removes the global-max dependency so per-tile exp/weighted-sum chains can
pipeline.  Inverse norms use exp(-0.5*ln(s)) so every ACT function in the
kernel (Identity/Relu/Ln/Exp) lives in one activation table - no reloads.

Scheduling: the potentials section is ACT/DVE-heavy but PE-light, so all PE
ops that consume its chain results (the 1/||ye|| broadcast, the q matmuls,
the weight broadcasts) are deferred into the NEXT batch's fc1/fc2 m-loops,
where a dense fp16 matmul block hides the chain latency (the PE is
in-order).

DoubleRow notes (discovered on this toolchain):
  - stationary operand must have >=32 active columns (M=32 minimum), so
    single-row reductions use a 32-wide duplicated stationary vector
  - PSUM output tiles must be allocated full [128, N] and sliced; a
    [32, N] PSUM tile silently produces garbage
  - operand layout is [128, 2, N] slices of [128, K, N] tiles (pair on the
    middle dim)
"""

import numpy as np
import ml_dtypes

import concourse.bass as bass
import concourse.bacc as bacc
import concourse.mybir as mybir
import concourse.tile as tile
from concourse.bass_utils import run_bass_kernel_spmd

B, T, IN, D = 32, 1024, 1536, 768
NCORES = 8
BPC = B // NCORES        # batches per core
NT = 512                 # token tile (matmul moving free dim)
NTT = T // NT            # token tiles per batch
KI = IN // 128           # 12 feature tiles of the 1536 dim
KD = D // 128            # 6 feature tiles of the 768 dim
KP = KD // 2             # 3 fp8 DoubleRow pairs of the 768 dim

F8 = mybir.dt.float8e4
F16 = mybir.dt.float16
F32 = mybir.dt.float32
AF = mybir.ActivationFunctionType
ALU = mybir.AluOpType
AX = mybir.AxisListType
DR = mybir.MatmulPerfMode.DoubleRow
NP8 = ml_dtypes.float8_e4m3


def build_nc(bpc: int = BPC) -> bass.Bass:
    nc = bacc.Bacc()

    xt = nc.declare_dram_parameter("xt", [bpc, IN, T], F16, isOutput=False)
    # w1 pre-arranged on host to the exact SBUF layout, m-chunk major:
    # w1m[p, m, k, j] = W1T[k*128+p, m*128+j]
    w1m = nc.declare_dram_parameter("w1m", [128, KI, KI, 128], F16,
                                    isOutput=False)
    w2t = nc.declare_dram_parameter("w2t", [IN, D], F16, isOutput=False)
    wet = nc.declare_dram_parameter("wet", [D, D], F8, isOutput=False)
    wxt = nc.declare_dram_parameter("wxt", [D, D], F8, isOutput=False)
    wyt = nc.declare_dram_parameter("wyt", [D, D], F8, isOutput=False)
    wrt = nc.declare_dram_parameter("wrt", [128, KD, 32], F8, isOutput=False)
    ones8d = nc.declare_dram_parameter("ones8", [128, 2, 32], F8,
                                       isOutput=False)
    b1d = nc.declare_dram_parameter("b1", [IN], F32, isOutput=False)
    b2d = nc.declare_dram_parameter("b2", [D], F32, isOutput=False)
    bed = nc.declare_dram_parameter("be", [D], F32, isOutput=False)
    bxd = nc.declare_dram_parameter("bx", [D], F32, isOutput=False)
    byd = nc.declare_dram_parameter("by", [D], F32, isOutput=False)
    # consts = [un_red_b*red_w0, red_w0, red_w1, 0]
    cst = nc.declare_dram_parameter("consts", [4], F32, isOutput=False)
    onesr16 = nc.declare_dram_parameter("onesr16", [1, 128], F16,
                                        isOutput=False)
    out = nc.declare_dram_parameter("out", [bpc, D], F32, isOutput=True)

    with tile.TileContext(nc) as tc:
        _body(nc, tc, bpc, xt, w1m, w2t, wet, wxt, wyt, wrt, ones8d,
              b1d, b2d, bed, bxd, byd, cst, onesr16, out)
    return nc


def _body(nc, tc, bpc, xt, w1m, w2t, wet, wxt, wyt, wrt, ones8d,
          b1d, b2d, bed, bxd, byd, cst, onesr16, out):
    with (
        tc.tile_pool(name="wpool", bufs=1) as wpool,
        tc.tile_pool(name="u16p", bufs=2) as u16p,
        tc.tile_pool(name="u8p", bufs=2) as u8p,
        tc.tile_pool(name="bat", bufs=1) as bat,
        tc.tile_pool(name="xp", bufs=2) as xp,
        tc.tile_pool(name="hp", bufs=1) as hp,
        tc.tile_pool(name="ep", bufs=1) as ep,
        tc.tile_pool(name="yp", bufs=2) as yp,
        tc.tile_pool(name="sqp", bufs=2) as sqp,
        tc.tile_pool(name="tmpp", bufs=2) as tmpp,
        tc.tile_pool(name="rows", bufs=1) as rows,
        tc.tile_pool(name="rtmp", bufs=2) as rtmp,
        tc.tile_pool(name="bc16p", bufs=2) as bc16p,
        tc.tile_pool(name="mmp", bufs=4, space="PSUM") as mmp,
        tc.tile_pool(name="rpp", bufs=2, space="PSUM") as rpp,
        tc.tile_pool(name="bcp", bufs=1, space="PSUM") as bcp,
        tc.tile_pool(name="stkp", bufs=1, space="PSUM") as stkp,
    ):
        # ---- persistent weights / constants ----
        # DMA order: x(b0,ti0) first, then w1 one m-chunk at a time (fc1's
        # m-block m needs only chunk m, so compute starts after ~2MB).
        first_xt = xp.tile([128, KI, NT], F16, tag="xt")
        x0r = xt[0].rearrange("(ko p) t -> p ko t", p=128)
        nc.sync.dma_start(first_xt, x0r[:, :, 0:NT])
        b1_sb = wpool.tile([128, KI], F32)
        nc.sync.dma_start(b1_sb, b1d.rearrange("(o p) -> p o", p=128))
        w1_sb = wpool.tile([128, KI, KI, 128], F16)
        for m in range(KI):
            nc.sync.dma_start(w1_sb[:, m], w1m[:, m])
        w2_sb = wpool.tile([128, KI, D], F16)
        w2r = w2t.rearrange("(ko p) m -> p ko m", p=128)
        for k in range(KI):
            nc.sync.dma_start(w2_sb[:, k, :], w2r[:, k, :])
        we_sb = wpool.tile([128, KD, D], F8)
        nc.sync.dma_start(we_sb, wet.rearrange("(ko p) m -> p ko m", p=128))
        wx_sb = wpool.tile([128, KD, D], F8)
        nc.sync.dma_start(wx_sb, wxt.rearrange("(ko p) m -> p ko m", p=128))
        wy_sb = wpool.tile([128, KD, D], F8)
        nc.sync.dma_start(wy_sb, wyt.rearrange("(ko p) m -> p ko m", p=128))
        wr_sb = wpool.tile([128, KD, 32], F8)
        nc.sync.dma_start(wr_sb, wrt[:, :, :])
        ones8_sb = wpool.tile([128, 2, 32], F8)
        nc.sync.dma_start(ones8_sb, ones8d[:, :, :])
        b2_sb = wpool.tile([128, KD], F32)
        nc.sync.dma_start(b2_sb, b2d.rearrange("(o p) -> p o", p=128))
        be_sb = wpool.tile([128, KD], F32)
        nc.sync.dma_start(be_sb, bed.rearrange("(o p) -> p o", p=128))
        bx_sb = wpool.tile([128, KD], F32)
        nc.sync.dma_start(bx_sb, bxd.rearrange("(o p) -> p o", p=128))
        by_sb = wpool.tile([128, KD], F32)
        nc.sync.dma_start(by_sb, byd.rearrange("(o p) -> p o", p=128))
        c_sb = wpool.tile([1, 4], F32)
        nc.sync.dma_start(c_sb, cst[None, :])
        onesr16_sb = wpool.tile([1, 128], F16)
        nc.sync.dma_start(onesr16_sb, onesr16[:, :])

        def alloc_batch(b):
            st = {"b": b}
            st["u16"] = u16p.tile([128, KD, T], F16, tag="u16", name=f"u16_{b}")
            st["u8"] = u8p.tile([128, KD, T], F8, tag="u8", name=f"u8_{b}")
            st["xe8"] = bat.tile([128, KD, T], F8, tag="xe8", name=f"xe_{b}")
            st["ybp"] = bat.tile([128, KD, NTT], F32, tag="ybp", name=f"yp_{b}")
            st["invx"] = rows.tile([1, T], F32, tag="invx", name=f"ix_{b}")
            st["scores"] = rows.tile([1, T], F32, tag="scores", name=f"sc_{b}")
            st["ewh"] = rows.tile([1, T], F16, tag="ewh", name=f"ew_{b}")
            st["smp"] = rows.tile([1, NTT], F32, tag="smp", name=f"sp_{b}")
            st["oacc"] = bat.tile([128, KD, NTT], F32, tag="oacc",
                                  name=f"oa_{b}")
            st["ye"] = [None] * NTT
            return st

        def fc1_part(st, ti, interleave=None):
            """interleave: optional fn(m) emitting deferred ops between
            m-blocks (hides the previous batch's reduction tail)."""
            b = st["b"]
            ns = slice(ti * NT, (ti + 1) * NT)
            if b == 0 and ti == 0:
                xt_sb = first_xt
            else:
                xt_sb = xp.tile([128, KI, NT], F16, tag="xt", name=f"xt{b}_{ti}")
                nc.sync.dma_start(
                    xt_sb,
                    xt[b].rearrange("(ko p) t -> p ko t", p=128)[:, :, ns])
            h = hp.tile([128, KI, NT], F16, tag="h", name=f"h{b}_{ti}")
            for m in range(KI):
                ps = mmp.tile([128, NT], F32, tag="mm")
                for k in range(KI):
                    nc.tensor.matmul(ps, w1_sb[:, m, k, :], xt_sb[:, k, :],
                                     start=(k == 0), stop=(k == KI - 1))
                nc.scalar.activation(h[:, m, :], ps, AF.Relu,
                                     bias=b1_sb[:, m:m + 1])
                if interleave is not None:
                    interleave(m)
            return h

        def fc2_part(st, ti, h, interleave=None):
            ns = slice(ti * NT, (ti + 1) * NT)
            for m in range(KD):
                ps = mmp.tile([128, NT], F32, tag="mm")
                for k in range(KI):
                    nc.tensor.matmul(ps, w2_sb[:, k, m * 128:(m + 1) * 128],
                                     h[:, k, :],
                                     start=(k == 0), stop=(k == KI - 1))
                nc.scalar.activation(st["u16"][:, m, ns], ps, AF.Identity,
                                     bias=b2_sb[:, m:m + 1])
                # second ACT read of the same PSUM emits the fp8 copy; keeps
                # the cast off the DVE queue and lets une start per-m
                nc.scalar.activation(st["u8"][:, m, ns], ps, AF.Identity,
                                     bias=b2_sb[:, m:m + 1])
                if interleave is not None:
                    interleave(m)

        def dr_mm(ps_out, w8, u8ap, tile_position=None):
            """3-pair fp8 DoubleRow accumulation over the 768 dim."""
            for j in range(KP):
                nc.tensor.matmul(ps_out, w8[:, 2 * j:2 * j + 2, :],
                                 u8ap[:, 2 * j:2 * j + 2, :],
                                 start=(j == 0), stop=(j == KP - 1),
                                 perf_mode=DR, tile_position=tile_position)

        def une_part(st, ti):
            b = st["b"]
            ns = slice(ti * NT, (ti + 1) * NT)
            e = ep.tile([128, KD, NT], F8, tag="e", name=f"e{b}_{ti}")
            for m in range(KD):
                ps = mmp.tile([128, NT], F32, tag="mm")
                dr_mm(ps, we_sb[:, :, m * 128:(m + 1) * 128],
                      st["u8"][:, :, ns])
                nc.scalar.activation(e[:, m, :], ps, AF.Relu,
                                     bias=be_sb[:, m:m + 1])
            st["e_%d" % ti] = e

        def unred_part(st, ti):
            # scores[ns] = rw0*un_pot = rw0*(Wr e) + rw0*br   (consts folded)
            ns = slice(ti * NT, (ti + 1) * NT)
            rps = rpp.tile([128, NT], F32, tag="row")
            dr_mm(rps[0:32, :], wr_sb, st["e_%d" % ti][:, :, :])
            nc.scalar.activation(st["scores"][:, ns], rps[0:1, :], AF.Identity,
                                 bias=c_sb[:, 0:1], scale=c_sb[:, 1:2])

        def pwx_part(st, ti):
            b = st["b"]
            ns = slice(ti * NT, (ti + 1) * NT)
            xe8 = st["xe8"]
            sq = sqp.tile([128, KD, NT], F8, tag="sq", name=f"sx{b}_{ti}")
            for m in range(KD):
                ps = mmp.tile([128, NT], F32, tag="mm")
                dr_mm(ps, wx_sb[:, :, m * 128:(m + 1) * 128],
                      st["u8"][:, :, ns])
                nc.scalar.activation(xe8[:, m, ns], ps, AF.Identity,
                                     bias=bx_sb[:, m:m + 1])
                if m % 3 == 2:
                    hs = slice(m - 2, m + 1)
                    eng = nc.vector if m == 2 else nc.gpsimd
                    eng.tensor_mul(sq[:, hs, :], xe8[:, hs, ns],
                                   xe8[:, hs, ns])
            st["sqx"] = sq

        def pwy_part(st, ti):
            b = st["b"]
            ns = slice(ti * NT, (ti + 1) * NT)
            ye = yp.tile([128, KD, NT], F16, tag="ye", name=f"ye{b}_{ti}")
            sq = sqp.tile([128, KD, NT], F8, tag="sq", name=f"sy{b}_{ti}")
            for m in range(KD):
                ps = mmp.tile([128, NT], F32, tag="mm")
                dr_mm(ps, wy_sb[:, :, m * 128:(m + 1) * 128],
                      st["u8"][:, :, ns])
                nc.scalar.activation(ye[:, m, :], ps, AF.Identity,
                                     bias=by_sb[:, m:m + 1])
                if m % 3 == 2:
                    hs = slice(m - 2, m + 1)
                    eng = nc.vector if m == 2 else nc.gpsimd
                    eng.tensor_mul(sq[:, hs, :], ye[:, hs, :],
                                   ye[:, hs, :])
            st["ye"][ti] = ye
            st["sqy"] = sq

        def sumsq_part(st, ti):
            # ssy and ssx in separate PSUM banks, both at partition base 0
            # (DoubleRow dst must start at partition 0 on this toolchain).
            # 1/sqrt via exp(-0.5*ln(s)): stays in the one loaded ACT table.
            ns = slice(ti * NT, (ti + 1) * NT)
            stk = stkp.tile([128, NT], F32, tag="stk")
            for j in range(KP):
                nc.tensor.matmul(stk[0:32, :], ones8_sb,
                                 st["sqy"][:, 2 * j:2 * j + 2, :],
                                 start=(j == 0), stop=(j == KP - 1),
                                 perf_mode=DR)
            rpx = rpp.tile([128, NT], F32, tag="row")
            for j in range(KP):
                nc.tensor.matmul(rpx[0:32, :], ones8_sb,
                                 st["sqx"][:, 2 * j:2 * j + 2, :],
                                 start=(j == 0), stop=(j == KP - 1),
                                 perf_mode=DR)
            t1 = rtmp.tile([1, NT], F32, tag="rt")
            nc.scalar.activation(t1, stk[0:1, :], AF.Ln)
            th = rtmp.tile([1, NT], F16, tag="rth")
            nc.scalar.activation(th, t1, AF.Exp, scale=-0.5)
            t0 = rtmp.tile([1, NT], F32, tag="rt")
            nc.scalar.activation(t0, rpx[0:1, :], AF.Ln)
            t0h = rtmp.tile([1, NT], F16, tag="sxr")
            nc.scalar.activation(t0h, t0, AF.Exp, scale=-0.5)
            # pre-scale by red_w1 so pass2 skips that op
            nc.vector.tensor_scalar_mul(st["invx"][:, ns], t0h, c_sb[:, 2:3])
            st["t1h_%d" % ti] = th

        def split_reduce(dst, tmp, tail):
            """dst[:, :, 0:1] = sum_X tmp.  In the exposed tail, split the
            reduction between DVE and ACT (accum_out) so it drains ~2x
            faster; in the pipeline one DVE op is cheaper."""
            if not tail:
                nc.vector.reduce_sum(dst, tmp, axis=AX.X)
                return
            nc.vector.reduce_sum(dst[:, 0:4], tmp[:, 0:4, :], axis=AX.X)
            for m in range(4, KD):
                nc.scalar.activation(tmp[:, m, :], tmp[:, m, :], AF.Identity,
                                     accum_out=dst[:, m, 0:1])

        def y_pe(st, ti, tail=False):
            # broadcast 1/||ye|| and reduce yn into ybar parts
            ye = st["ye"][ti]
            ivb = bcp.tile([128, NT], F32, tag="bc")
            nc.tensor.matmul(ivb, onesr16_sb, st["t1h_%d" % ti],
                             start=True, stop=True)
            ivb16 = bc16p.tile([128, 1, NT], F16, tag="bc16")
            nc.scalar.activation(ivb16[:, 0, :], ivb, AF.Identity)
            tmp = tmpp.tile([128, KD, NT], F16, tag="tmp")
            nc.vector.tensor_mul(tmp, ye,
                                 ivb16.broadcast_to([128, KD, NT]))
            split_reduce(st["ybp"][:, :, ti:ti + 1], tmp, tail)
            if ti == NTT - 1:
                b = st["b"]
                # 32-wide duplicated fp8 stationary for the q matmul
                ybar8 = bat.tile([128, KD, 32], F8, tag="ybar", name=f"yb{b}")
                ybf = bat.tile([128, KD, 1], F32, tag="ybf", name=f"yf{b}")
                nc.vector.tensor_add(ybf, st["ybp"][:, :, 0:1],
                                     st["ybp"][:, :, 1:2])
                nc.vector.tensor_scalar_mul(ybf, ybf, 1.0 / T)
                nc.vector.tensor_copy(ybar8, ybf.broadcast_to([128, KD, 32]))
                st["ybar8"] = ybar8

        def pass2_q(st, ti):
            # q = xe . ybar ; scores[ns] += rw1 * q * invx ; exp -> ewh
            ns = slice(ti * NT, (ti + 1) * NT)
            qps = rpp.tile([128, NT], F32, tag="row")
            dr_mm(qps[0:32, :], st["ybar8"], st["xe8"][:, :, ns])
            s0 = rtmp.tile([1, NT], F32, tag="rt")
            nc.vector.tensor_mul(s0, qps[0:1, :], st["invx"][:, ns])
            nc.vector.tensor_add(s0, s0, st["scores"][:, ns])
            # no max subtraction: scores are bounded (~3), exp is safe
            nc.scalar.activation(st["ewh"][:, ns], s0, AF.Exp)
            nc.vector.reduce_sum(st["smp"][:, ti:ti + 1], st["ewh"][:, ns],
                                 axis=AX.X)

        def pass2_w(st, ti, tail=False):
            # oacc[:, :, ti] = sum_{t in ns} ewh[t] * u16[:, t]
            ns = slice(ti * NT, (ti + 1) * NT)
            wbc = bcp.tile([128, NT], F32, tag="bc")
            nc.tensor.matmul(wbc, onesr16_sb, st["ewh"][:, ns],
                             start=True, stop=True)
            wbc16 = bc16p.tile([128, 1, NT], F16, tag="bc16")
            nc.scalar.activation(wbc16[:, 0, :], wbc, AF.Identity)
            tmp = tmpp.tile([128, KD, NT], F16, tag="tmp")
            nc.vector.tensor_mul(tmp, st["u16"][:, :, ns],
                                 wbc16.broadcast_to([128, KD, NT]))
            split_reduce(st["oacc"][:, :, ti:ti + 1], tmp, tail)

        def pass2_fin(st):
            b = st["b"]
            sm = rows.tile([1, 1], F32, tag="sm", name=f"sm{b}")
            nc.vector.tensor_add(sm, st["smp"][:, 0:1], st["smp"][:, 1:2])
            nc.vector.reciprocal(sm, sm)
            smh = rows.tile([1, 1], F16, tag="smh", name=f"sh{b}")
            nc.vector.tensor_copy(smh, sm)
            smb = bcp.tile([128, 1], F32, tag="bc")
            nc.tensor.matmul(smb, onesr16_sb, smh, start=True, stop=True)
            ofin = bat.tile([128, KD, 1], F32, tag="ofin", name=f"of{b}")
            nc.vector.tensor_add(ofin, st["oacc"][:, :, 0:1],
                                 st["oacc"][:, :, 1:2])
            nc.vector.tensor_scalar_mul(ofin, ofin, smb)
            nc.sync.dma_start(out[b].rearrange("(mo p) -> p mo", p=128),
                              ofin[:, :, 0])

        def il_fc1(prev):
            """prev's reduction tail, spread across fc1's 12 m-blocks."""
            if prev is None:
                return None

            def f(m):
                if m == 1:
                    y_pe(prev, 0)
                elif m == 3:
                    y_pe(prev, 1)     # includes the ybar8 chain
                elif m == 6:
                    pass2_q(prev, 0)
                elif m == 8:
                    pass2_q(prev, 1)
                elif m == 10:
                    pass2_w(prev, 0)
            return f

        def il_fc2(prev):
            if prev is None:
                return None

            def f(m):
                if m == 1:
                    pass2_w(prev, 1)
                elif m == 3:
                    pass2_fin(prev)
            return f

        prev = None
        for b in range(bpc):
            st = alloc_batch(b)
            h0 = fc1_part(st, 0, interleave=il_fc1(prev))
            fc2_part(st, 0, h0, interleave=il_fc2(prev))
            h1 = fc1_part(st, 1)
            fc2_part(st, 1, h1)
            une_part(st, 0)
            pwx_part(st, 0)
            pwy_part(st, 0)
            unred_part(st, 0)
            sumsq_part(st, 0)
            une_part(st, 1)
            if b == bpc - 1:
                # last batch: ti0's ybar contribution runs under ti1's
                # potentials so only ti1's short chain is exposed in the tail
                y_pe(st, 0, tail=True)
            pwx_part(st, 1)
            pwy_part(st, 1)
            unred_part(st, 1)
            sumsq_part(st, 1)
            prev = st
        y_pe(prev, 1, tail=True)
        pass2_q(prev, 0)
        pass2_q(prev, 1)
        pass2_w(prev, 0, tail=True)
        pass2_w(prev, 1, tail=True)
        pass2_fin(prev)


_CACHE = {}


def _patch_act_tables():
    """Steer every activation to the one table that contains all the funcs
    this kernel uses (Identity/Relu/Ln/Exp), so the ACT engine loads its
    piecewise-polynomial table once instead of ping-ponging between the
    exp and ln tables every batch (1.3us per reload, on the critical path).
    Table order (= act_func_set_id) is preserved; only the func sets the
    placement pass sees are filtered."""
    import concourse.bacc as bacc_mod
    if getattr(bacc_mod, "_act_tables_patched", False):
        return
    orig = bacc_mod.get_activation_tables
    keep = "natural_log_exp_and_others"
    mine = {AF.Identity, AF.Relu, AF.Ln, AF.Exp, AF.Copy, AF.Square}

    def patched(arch):
        tabs = orig(arch)
        if keep not in tabs or not mine <= tabs[keep]:
            return tabs
        return {name: (funcs if name == keep else funcs - mine)
                for name, funcs in tabs.items()}

    bacc_mod.get_activation_tables = patched
    bacc_mod._act_tables_patched = True


def _get_nc():
    if "nc" not in _CACHE:
        _patch_act_tables()
        nc = build_nc(BPC)
        nc.finalize()
        _CACHE["nc"] = nc
    return _CACHE["nc"]


def make_in_maps(x, fc1_w, fc1_b, fc2_w, fc2_b, un_emb_w, un_emb_b,
                 un_red_w, un_red_b, pw_x_w, pw_x_b, pw_y_w, pw_y_b, red_w):
    w1t = np.ascontiguousarray(fc1_w.T).astype(np.float16)    # [IN(ki), IN(m)]
    # w1m[p, m, k, j] = w1t[k*128+p, m*128+j]
    w1m = np.ascontiguousarray(
        w1t.reshape(KI, 128, KI, 128).transpose(1, 2, 0, 3))
    wr_col = np.ascontiguousarray(un_red_w.T).reshape(KD, 128).transpose(1, 0)
    shared = {
        "w1m": w1m,
        "w2t": np.ascontiguousarray(fc2_w.T).astype(np.float16),
        "wet": np.ascontiguousarray(un_emb_w.T).astype(NP8),
        "wxt": np.ascontiguousarray(pw_x_w.T).astype(NP8),
        "wyt": np.ascontiguousarray(pw_y_w.T).astype(NP8),
        "wrt": np.ascontiguousarray(
            np.repeat(wr_col[:, :, None], 32, axis=2)).astype(NP8),
        "ones8": np.ones([128, 2, 32], NP8),
        "b1": np.asarray(fc1_b, np.float32),
        "b2": np.asarray(fc2_b, np.float32),
        "be": np.asarray(un_emb_b, np.float32),
        "bx": np.asarray(pw_x_b, np.float32),
        "by": np.asarray(pw_y_b, np.float32),
        "consts": np.array([un_red_b[0] * red_w[0], red_w[0], red_w[1], 0.0],
                           np.float32),
        "onesr16": np.ones([1, 128], np.float16),
    }
    in_maps = []
    for c in range(NCORES):
        xs = np.ascontiguousarray(
            x[c * BPC:(c + 1) * BPC].transpose(0, 2, 1)).astype(np.float16)
        in_maps.append({"xt": xs, **shared})
    return in_maps


def kernel(**inputs) -> np.ndarray:
    inputs = {k: np.asarray(v) for k, v in inputs.items()}
    nc = _get_nc()
    in_maps = make_in_maps(**inputs)
    res = run_bass_kernel_spmd(nc, in_maps, core_ids=list(range(NCORES)))
    return np.concatenate([res.results[c]["out"] for c in range(NCORES)], axis=0)


# revision 47
# speedup vs baseline: 1.0575x; 1.0575x over previous
"""Trainium2 Bass kernel for FGAEmbedder (B=32, T=1024, IN=1536, D=768).

Math (identical to the reference up to float reassociation):
    h  = relu(x @ W1^T + b1)           [B,T,IN]
    u  = h @ W2^T + b2                 [B,T,D]
    e  = relu(u @ We^T + be)
    un = e @ Wr^T + br                 [B,T]
    xe = u @ Wx^T + bx ; ye = u @ Wy^T + by
    pw[t] = mean_s cos(xe[t], ye[s]) = (xe[t] . ybar) / ||xe[t]||,
            ybar = mean_s ye[s]/||ye[s]||      (the TxT matrix never exists)
    out = sum_t softmax(rw0*un + rw1*pw)[t] * u[t]

Sharding: data-parallel over batch, 4 batches per core, weights replicated,
no collectives.  Activations are feature-major on chip ([feat, tok]).

Precision strategy: fc1/fc2 run in fp16 (u directly forms the output), but
every matmul that only feeds the softmax scores runs in fp8e4 with DoubleRow
double-pumping (2 contraction rows per cycle): une, un_red, pw_x, pw_y, the
sum-of-squares row reductions, and the q = xe.ybar matmul.  Score errors are
diluted through the softmax; measured end-to-end rel err ~7e-3 vs the 2e-2
gate.  Softmax skips the max subtraction (scores are bounded ~3), which
removes the global-max dependency so per-tile exp/weighted-sum chains can
pipeline.  Inverse norms use exp(-0.5*ln(s)) so every ACT function in the
kernel (Identity/Relu/Ln/Exp) lives in one activation table - no reloads.

Scheduling: the potentials section is ACT/DVE-heavy but PE-light, so all PE
ops that consume its chain results (the 1/||ye|| broadcast, the q matmuls,
the weight broadcasts) are deferred into the NEXT batch's fc1/fc2 m-loops,
where a dense fp16 matmul block hides the chain latency (the PE is
in-order).

DoubleRow notes (discovered on this toolchain):
  - stationary operand must have >=32 active columns (M=32 minimum), so
    single-row reductions use a 32-wide duplicated stationary vector
  - PSUM output tiles must be allocated full [128, N] and sliced; a
    [32, N] PSUM tile silently produces garbage
  - operand layout is [128, 2, N] slices of [128, K, N] tiles (pair on the
    middle dim)
"""

import numpy as np
import ml_dtypes

import concourse.bass as bass
import concourse.bacc as bacc
import concourse.mybir as mybir
import concourse.tile as tile
from concourse.bass_utils import run_bass_kernel_spmd

B, T, IN, D = 32, 1024, 1536, 768
NCORES = 8
BPC = B // NCORES        # batches per core
NT = 512                 # token tile (matmul moving free dim)
NTT = T // NT            # token tiles per batch
KI = IN // 128           # 12 feature tiles of the 1536 dim
KD = D // 128            # 6 feature tiles of the 768 dim
KP = KD // 2             # 3 fp8 DoubleRow pairs of the 768 dim

F8 = mybir.dt.float8e4
F16 = mybir.dt.float16
F32 = mybir.dt.float32
AF = mybir.ActivationFunctionType
ALU = mybir.AluOpType
AX = mybir.AxisListType
DR = mybir.MatmulPerfMode.DoubleRow
NP8 = ml_dtypes.float8_e4m3


def build_nc(bpc: int = BPC) -> bass.Bass:
    nc = bacc.Bacc()

    xt = nc.declare_dram_parameter("xt", [bpc, IN, T], F16, isOutput=False)
    # w1 pre-arranged on host to the exact SBUF layout, m-chunk major:
    # w1m[p, m, k, j] = W1T[k*128+p, m*128+j]
    w1m = nc.declare_dram_parameter("w1m", [128, KI, KI, 128], F16,
                                    isOutput=False)
    w2t = nc.declare_dram_parameter("w2t", [IN, D], F16, isOutput=False)
    wet = nc.declare_dram_parameter("wet", [D, D], F8, isOutput=False)
    wxt = nc.declare_dram_parameter("wxt", [D, D], F8, isOutput=False)
    wyt = nc.declare_dram_parameter("wyt", [D, D], F8, isOutput=False)
    wrt = nc.declare_dram_parameter("wrt", [128, KD, 32], F8, isOutput=False)
    ones8d = nc.declare_dram_parameter("ones8", [128, 2, 32], F8,
                                       isOutput=False)
    b1d = nc.declare_dram_parameter("b1", [IN], F32, isOutput=False)
    b2d = nc.declare_dram_parameter("b2", [D], F32, isOutput=False)
    bed = nc.declare_dram_parameter("be", [D], F32, isOutput=False)
    bxd = nc.declare_dram_parameter("bx", [D], F32, isOutput=False)
    byd = nc.declare_dram_parameter("by", [D], F32, isOutput=False)
    # consts = [un_red_b*red_w0, red_w0, red_w1, 0]
    cst = nc.declare_dram_parameter("consts", [4], F32, isOutput=False)
    onesr16 = nc.declare_dram_parameter("onesr16", [1, 128], F16,
                                        isOutput=False)
    out = nc.declare_dram_parameter("out", [bpc, D], F32, isOutput=True)

    with tile.TileContext(nc) as tc:
        _body(nc, tc, bpc, xt, w1m, w2t, wet, wxt, wyt, wrt, ones8d,
              b1d, b2d, bed, bxd, byd, cst, onesr16, out)
    return nc


def _body(nc, tc, bpc, xt, w1m, w2t, wet, wxt, wyt, wrt, ones8d,
          b1d, b2d, bed, bxd, byd, cst, onesr16, out):
    with (
        tc.tile_pool(name="wpool", bufs=1) as wpool,
        tc.tile_pool(name="u16p", bufs=2) as u16p,
        tc.tile_pool(name="u8p", bufs=2) as u8p,
        tc.tile_pool(name="bat", bufs=1) as bat,
        tc.tile_pool(name="xp", bufs=2) as xp,
        tc.tile_pool(name="hp", bufs=1) as hp,
        tc.tile_pool(name="ep", bufs=1) as ep,
        tc.tile_pool(name="yp", bufs=2) as yp,
        tc.tile_pool(name="sqp", bufs=2) as sqp,
        tc.tile_pool(name="tmpp", bufs=2) as tmpp,
        tc.tile_pool(name="rows", bufs=1) as rows,
        tc.tile_pool(name="rtmp", bufs=2) as rtmp,
        tc.tile_pool(name="bc16p", bufs=2) as bc16p,
        tc.tile_pool(name="mmp", bufs=5, space="PSUM") as mmp,
        tc.tile_pool(name="rpp", bufs=1, space="PSUM") as rpp,
        tc.tile_pool(name="bcp", bufs=1, space="PSUM") as bcp,
        tc.tile_pool(name="stkp", bufs=1, space="PSUM") as stkp,
    ):
        # ---- persistent weights / constants ----
        # DMA order: first half of x(b0,ti0), then w1 m-chunk 0, then the
        # rest - fc1's first m-block starts after ~1.2MB instead of 6MB.
        NH = NT // 2
        first_xt = xp.tile([128, KI, NT], F16, tag="xt")
        x0r = xt[0].rearrange("(ko p) t -> p ko t", p=128)
        nc.sync.dma_start(first_xt[:, :, 0:NH], x0r[:, :, 0:NH])
        w1_sb = wpool.tile([128, KI, KI, 128], F16)
        nc.sync.dma_start(w1_sb[:, 0], w1m[:, 0])
        nc.sync.dma_start(first_xt[:, :, NH:NT], x0r[:, :, NH:NT])
        b1_sb = wpool.tile([128, KI], F32)
        nc.sync.dma_start(b1_sb, b1d.rearrange("(o p) -> p o", p=128))
        for m in range(1, KI):
            nc.sync.dma_start(w1_sb[:, m], w1m[:, m])
        w2_sb = wpool.tile([128, KI, D], F16)
        w2r = w2t.rearrange("(ko p) m -> p ko m", p=128)
        for k in range(KI):
            nc.sync.dma_start(w2_sb[:, k, :], w2r[:, k, :])
        we_sb = wpool.tile([128, KD, D], F8)
        nc.sync.dma_start(we_sb, wet.rearrange("(ko p) m -> p ko m", p=128))
        wx_sb = wpool.tile([128, KD, D], F8)
        nc.sync.dma_start(wx_sb, wxt.rearrange("(ko p) m -> p ko m", p=128))
        wy_sb = wpool.tile([128, KD, D], F8)
        nc.sync.dma_start(wy_sb, wyt.rearrange("(ko p) m -> p ko m", p=128))
        wr_sb = wpool.tile([128, KD, 32], F8)
        nc.sync.dma_start(wr_sb, wrt[:, :, :])
        ones8_sb = wpool.tile([128, 2, 32], F8)
        nc.sync.dma_start(ones8_sb, ones8d[:, :, :])
        b2_sb = wpool.tile([128, KD], F32)
        nc.sync.dma_start(b2_sb, b2d.rearrange("(o p) -> p o", p=128))
        be_sb = wpool.tile([128, KD], F32)
        nc.sync.dma_start(be_sb, bed.rearrange("(o p) -> p o", p=128))
        bx_sb = wpool.tile([128, KD], F32)
        nc.sync.dma_start(bx_sb, bxd.rearrange("(o p) -> p o", p=128))
        by_sb = wpool.tile([128, KD], F32)
        nc.sync.dma_start(by_sb, byd.rearrange("(o p) -> p o", p=128))
        c_sb = wpool.tile([1, 4], F32)
        nc.sync.dma_start(c_sb, cst[None, :])
        onesr16_sb = wpool.tile([1, 128], F16)
        nc.sync.dma_start(onesr16_sb, onesr16[:, :])

        def alloc_batch(b):
            st = {"b": b}
            st["u16"] = u16p.tile([128, KD, T], F16, tag="u16", name=f"u16_{b}")
            st["u8"] = u8p.tile([128, KD, T], F8, tag="u8", name=f"u8_{b}")
            st["xe8"] = bat.tile([128, KD, T], F8, tag="xe8", name=f"xe_{b}")
            st["ybp"] = bat.tile([128, KD, NTT], F32, tag="ybp", name=f"yp_{b}")
            st["invx"] = rows.tile([1, T], F32, tag="invx", name=f"ix_{b}")
            st["scores"] = rows.tile([1, T], F32, tag="scores", name=f"sc_{b}")
            st["ewh"] = rows.tile([1, T], F16, tag="ewh", name=f"ew_{b}")
            st["smp"] = rows.tile([1, NTT], F32, tag="smp", name=f"sp_{b}")
            st["oacc"] = bat.tile([128, KD, NTT], F32, tag="oacc",
                                  name=f"oa_{b}")
            st["ye"] = [None] * NTT
            return st

        def fc1_part(st, ti, interleave=None):
            """interleave: optional fn(m) emitting deferred ops between
            m-blocks (hides the previous batch's reduction tail)."""
            b = st["b"]
            ns = slice(ti * NT, (ti + 1) * NT)
            halves = None
            if b == 0 and ti == 0:
                xt_sb = first_xt
                halves = (first_xt[:, :, 0:NH], first_xt[:, :, NH:NT])
            else:
                xt_sb = xp.tile([128, KI, NT], F16, tag="xt", name=f"xt{b}_{ti}")
                nc.sync.dma_start(
                    xt_sb,
                    xt[b].rearrange("(ko p) t -> p ko t", p=128)[:, :, ns])
            h = hp.tile([128, KI, NT], F16, tag="h", name=f"h{b}_{ti}")
            for m in range(KI):
                ps = mmp.tile([128, NT], F32, tag="mm")
                if halves is not None:
                    # startup: col-halved accumulation groups so compute
                    # starts as soon as the first half-tile of x lands
                    for hx, xh in enumerate(halves):
                        cs = slice(hx * (NT // 2), (hx + 1) * (NT // 2))
                        for k in range(KI):
                            nc.tensor.matmul(ps[:, cs], w1_sb[:, m, k, :],
                                             xh[:, k, :],
                                             start=(k == 0),
                                             stop=(k == KI - 1))
                else:
                    for k in range(KI):
                        nc.tensor.matmul(ps, w1_sb[:, m, k, :], xt_sb[:, k, :],
                                         start=(k == 0), stop=(k == KI - 1))
                nc.scalar.activation(h[:, m, :], ps, AF.Relu,
                                     bias=b1_sb[:, m:m + 1])
                if interleave is not None:
                    interleave(m)
            return h

        def fc2_part(st, ti, h, interleave=None):
            ns = slice(ti * NT, (ti + 1) * NT)
            for m in range(KD):
                ps = mmp.tile([128, NT], F32, tag="mm")
                for k in range(KI):
                    nc.tensor.matmul(ps, w2_sb[:, k, m * 128:(m + 1) * 128],
                                     h[:, k, :],
                                     start=(k == 0), stop=(k == KI - 1))
                nc.scalar.activation(st["u16"][:, m, ns], ps, AF.Identity,
                                     bias=b2_sb[:, m:m + 1])
                # second ACT read of the same PSUM emits the fp8 copy; keeps
                # the cast off the DVE queue and lets une start per-m
                nc.scalar.activation(st["u8"][:, m, ns], ps, AF.Identity,
                                     bias=b2_sb[:, m:m + 1])
                if interleave is not None:
                    interleave(m)

        def dr_mm(ps_out, w8, u8ap, tile_position=None):
            """3-pair fp8 DoubleRow accumulation over the 768 dim."""
            for j in range(KP):
                nc.tensor.matmul(ps_out, w8[:, 2 * j:2 * j + 2, :],
                                 u8ap[:, 2 * j:2 * j + 2, :],
                                 start=(j == 0), stop=(j == KP - 1),
                                 perf_mode=DR, tile_position=tile_position)

        def une_part(st, ti):
            b = st["b"]
            ns = slice(ti * NT, (ti + 1) * NT)
            e = ep.tile([128, KD, NT], F8, tag="e", name=f"e{b}_{ti}")
            for m in range(KD):
                ps = mmp.tile([128, NT], F32, tag="mm")
                dr_mm(ps, we_sb[:, :, m * 128:(m + 1) * 128],
                      st["u8"][:, :, ns])
                if m in (1, 4):
                    # ACT saturates on PSUM drains in the potentials phases;
                    # DVE takes every third one (relu via add+max)
                    nc.vector.tensor_scalar(e[:, m, :], ps,
                                            be_sb[:, m:m + 1], 0.0,
                                            ALU.add, ALU.max)
                else:
                    nc.scalar.activation(e[:, m, :], ps, AF.Relu,
                                         bias=be_sb[:, m:m + 1])
            st["e_%d" % ti] = e

        def unred_part(st, ti):
            # scores[ns] = rw0*un_pot = rw0*(Wr e) + rw0*br   (consts folded)
            ns = slice(ti * NT, (ti + 1) * NT)
            rps = rpp.tile([128, NT], F32, tag="row")
            dr_mm(rps[0:32, :], wr_sb, st["e_%d" % ti][:, :, :])
            nc.scalar.activation(st["scores"][:, ns], rps[0:1, :], AF.Identity,
                                 bias=c_sb[:, 0:1], scale=c_sb[:, 1:2])

        def pwx_part(st, ti):
            b = st["b"]
            pool_sq = b < bpc - 1   # exposed tail: keep sq off the slow Pool
            ns = slice(ti * NT, (ti + 1) * NT)
            xe8 = st["xe8"]
            sq = sqp.tile([128, KD, NT], F8, tag="sq", name=f"sx{b}_{ti}")
            for m in range(KD):
                ps = mmp.tile([128, NT], F32, tag="mm")
                dr_mm(ps, wx_sb[:, :, m * 128:(m + 1) * 128],
                      st["u8"][:, :, ns])
                if m in (1, 4):
                    nc.vector.tensor_scalar_add(xe8[:, m, ns], ps,
                                                bx_sb[:, m:m + 1])
                else:
                    nc.scalar.activation(xe8[:, m, ns], ps, AF.Identity,
                                         bias=bx_sb[:, m:m + 1])
                if m % 3 == 2:
                    hs = slice(m - 2, m + 1)
                    eng = nc.vector if (m == 2 or not pool_sq) else nc.gpsimd
                    eng.tensor_mul(sq[:, hs, :], xe8[:, hs, ns],
                                   xe8[:, hs, ns])
            st["sqx"] = sq

        def pwy_part(st, ti):
            b = st["b"]
            pool_sq = b < bpc - 1
            ns = slice(ti * NT, (ti + 1) * NT)
            ye = yp.tile([128, KD, NT], F16, tag="ye", name=f"ye{b}_{ti}")
            sq = sqp.tile([128, KD, NT], F8, tag="sq", name=f"sy{b}_{ti}")
            for m in range(KD):
                ps = mmp.tile([128, NT], F32, tag="mm")
                dr_mm(ps, wy_sb[:, :, m * 128:(m + 1) * 128],
                      st["u8"][:, :, ns])
                if m in (1, 4):
                    nc.vector.tensor_scalar_add(ye[:, m, :], ps,
                                                by_sb[:, m:m + 1])
                else:
                    nc.scalar.activation(ye[:, m, :], ps, AF.Identity,
                                         bias=by_sb[:, m:m + 1])
                if m % 3 == 2:
                    hs = slice(m - 2, m + 1)
                    eng = nc.vector if (m == 2 or not pool_sq) else nc.gpsimd
                    eng.tensor_mul(sq[:, hs, :], ye[:, hs, :],
                                   ye[:, hs, :])
            st["ye"][ti] = ye
            st["sqy"] = sq

        def sumsq_part(st, ti):
            # ssy and ssx in separate PSUM banks, both at partition base 0
            # (DoubleRow dst must start at partition 0 on this toolchain).
            # 1/sqrt via exp(-0.5*ln(s)): stays in the one loaded ACT table.
            ns = slice(ti * NT, (ti + 1) * NT)
            stk = stkp.tile([128, NT], F32, tag="stk")
            for j in range(KP):
                nc.tensor.matmul(stk[0:32, :], ones8_sb,
                                 st["sqy"][:, 2 * j:2 * j + 2, :],
                                 start=(j == 0), stop=(j == KP - 1),
                                 perf_mode=DR)
            rpx = rpp.tile([128, NT], F32, tag="row")
            for j in range(KP):
                nc.tensor.matmul(rpx[0:32, :], ones8_sb,
                                 st["sqx"][:, 2 * j:2 * j + 2, :],
                                 start=(j == 0), stop=(j == KP - 1),
                                 perf_mode=DR)
            t1 = rtmp.tile([1, NT], F32, tag="rt")
            nc.scalar.activation(t1, stk[0:1, :], AF.Ln)
            th = rtmp.tile([1, NT], F16, tag="rth")
            nc.scalar.activation(th, t1, AF.Exp, scale=-0.5)
            t0 = rtmp.tile([1, NT], F32, tag="rt")
            nc.scalar.activation(t0, rpx[0:1, :], AF.Ln)
            t0h = rtmp.tile([1, NT], F16, tag="sxr")
            nc.scalar.activation(t0h, t0, AF.Exp, scale=-0.5)
            # pre-scale by red_w1 so pass2 skips that op
            nc.vector.tensor_scalar_mul(st["invx"][:, ns], t0h, c_sb[:, 2:3])
            st["t1h_%d" % ti] = th

        def split_reduce(dst, tmp, tail):
            """dst[:, :, 0:1] = sum_X tmp.  In the exposed tail, split the
            reduction between DVE and ACT (accum_out) so it drains ~2x
            faster; in the pipeline one DVE op is cheaper."""
            if not tail:
                nc.vector.reduce_sum(dst, tmp, axis=AX.X)
                return
            nc.vector.reduce_sum(dst[:, 0:4], tmp[:, 0:4, :], axis=AX.X)
            for m in range(4, KD):
                nc.scalar.activation(tmp[:, m, :], tmp[:, m, :], AF.Identity,
                                     accum_out=dst[:, m, 0:1])

        def y_pe(st, ti, tail=False):
            # broadcast 1/||ye|| and reduce yn into ybar parts
            ye = st["ye"][ti]
            ivb = bcp.tile([128, NT], F32, tag="bc")
            nc.tensor.matmul(ivb, onesr16_sb, st["t1h_%d" % ti],
                             start=True, stop=True)
            ivb16 = bc16p.tile([128, 1, NT], F16, tag="bc16")
            nc.scalar.activation(ivb16[:, 0, :], ivb, AF.Identity)
            tmp = tmpp.tile([128, KD, NT], F16, tag="tmp")
            nc.vector.tensor_mul(tmp, ye,
                                 ivb16.broadcast_to([128, KD, NT]))
            split_reduce(st["ybp"][:, :, ti:ti + 1], tmp, tail)
            if ti == NTT - 1:
                b = st["b"]
                # 32-wide duplicated fp8 stationary for the q matmul
                ybar8 = bat.tile([128, KD, 32], F8, tag="ybar", name=f"yb{b}")
                ybf = bat.tile([128, KD, 1], F32, tag="ybf", name=f"yf{b}")
                nc.vector.tensor_add(ybf, st["ybp"][:, :, 0:1],
                                     st["ybp"][:, :, 1:2])
                nc.vector.tensor_scalar_mul(ybf, ybf, 1.0 / T)
                nc.vector.tensor_copy(ybar8, ybf.broadcast_to([128, KD, 32]))
                st["ybar8"] = ybar8

        def pass2_q(st, ti):
            # q = xe . ybar ; scores[ns] += rw1 * q * invx ; exp -> ewh
            ns = slice(ti * NT, (ti + 1) * NT)
            qps = rpp.tile([128, NT], F32, tag="row")
            dr_mm(qps[0:32, :], st["ybar8"], st["xe8"][:, :, ns])
            s0 = rtmp.tile([1, NT], F32, tag="rt")
            nc.vector.tensor_mul(s0, qps[0:1, :], st["invx"][:, ns])
            nc.vector.tensor_add(s0, s0, st["scores"][:, ns])
            # no max subtraction: scores are bounded (~3), exp is safe
            nc.scalar.activation(st["ewh"][:, ns], s0, AF.Exp)
            nc.vector.reduce_sum(st["smp"][:, ti:ti + 1], st["ewh"][:, ns],
                                 axis=AX.X)

        def pass2_w(st, ti, tail=False):
            # oacc[:, :, ti] = sum_{t in ns} ewh[t] * u16[:, t]
            ns = slice(ti * NT, (ti + 1) * NT)
            wbc = bcp.tile([128, NT], F32, tag="bc")
            nc.tensor.matmul(wbc, onesr16_sb, st["ewh"][:, ns],
                             start=True, stop=True)
            wbc16 = bc16p.tile([128, 1, NT], F16, tag="bc16")
            nc.scalar.activation(wbc16[:, 0, :], wbc, AF.Identity)
            tmp = tmpp.tile([128, KD, NT], F16, tag="tmp")
            nc.vector.tensor_mul(tmp, st["u16"][:, :, ns],
                                 wbc16.broadcast_to([128, KD, NT]))
            split_reduce(st["oacc"][:, :, ti:ti + 1], tmp, tail)

        def pass2_fin(st):
            b = st["b"]
            sm = rows.tile([1, 1], F32, tag="sm", name=f"sm{b}")
            nc.vector.tensor_add(sm, st["smp"][:, 0:1], st["smp"][:, 1:2])
            nc.vector.reciprocal(sm, sm)
            smh = rows.tile([1, 1], F16, tag="smh", name=f"sh{b}")
            nc.vector.tensor_copy(smh, sm)
            smb = bcp.tile([128, 1], F32, tag="bc")
            nc.tensor.matmul(smb, onesr16_sb, smh, start=True, stop=True)
            ofin = bat.tile([128, KD, 1], F32, tag="ofin", name=f"of{b}")
            nc.vector.tensor_add(ofin, st["oacc"][:, :, 0:1],
                                 st["oacc"][:, :, 1:2])
            nc.vector.tensor_scalar_mul(ofin, ofin, smb)
            nc.sync.dma_start(out[b].rearrange("(mo p) -> p mo", p=128),
                              ofin[:, :, 0])

        def il_fc1(prev):
            """prev's reduction tail, spread across fc1's 12 m-blocks."""
            if prev is None:
                return None

            def f(m):
                if m == 1:
                    y_pe(prev, 0)
                elif m == 3:
                    y_pe(prev, 1)     # includes the ybar8 chain
                elif m == 6:
                    pass2_q(prev, 0)
                elif m == 8:
                    pass2_q(prev, 1)
                elif m == 10:
                    pass2_w(prev, 0)
            return f

        def il_fc2(prev):
            if prev is None:
                return None

            def f(m):
                if m == 1:
                    pass2_w(prev, 1)
                elif m == 3:
                    pass2_fin(prev)
            return f

        prev = None
        for b in range(bpc):
            st = alloc_batch(b)
            h0 = fc1_part(st, 0, interleave=il_fc1(prev))
            fc2_part(st, 0, h0, interleave=il_fc2(prev))
            h1 = fc1_part(st, 1)
            fc2_part(st, 1, h1)
            une_part(st, 0)
            pwx_part(st, 0)
            pwy_part(st, 0)
            unred_part(st, 0)
            sumsq_part(st, 0)
            une_part(st, 1)
            if b == bpc - 1:
                # last batch: ti0's ybar contribution runs under ti1's
                # potentials so only ti1's short chain is exposed in the tail
                y_pe(st, 0, tail=True)
            pwx_part(st, 1)
            pwy_part(st, 1)
            unred_part(st, 1)
            sumsq_part(st, 1)
            prev = st
        y_pe(prev, 1, tail=True)
        pass2_q(prev, 0)
        pass2_q(prev, 1)
        pass2_w(prev, 0, tail=True)
        pass2_w(prev, 1, tail=True)
        pass2_fin(prev)


_CACHE = {}


def _patch_act_tables():
    """Steer every activation to the one table that contains all the funcs
    this kernel uses (Identity/Relu/Ln/Exp), so the ACT engine loads its
    piecewise-polynomial table once instead of ping-ponging between the
    exp and ln tables every batch (1.3us per reload, on the critical path).
    Table order (= act_func_set_id) is preserved; only the func sets the
    placement pass sees are filtered."""
    import concourse.bacc as bacc_mod
    if getattr(bacc_mod, "_act_tables_patched", False):
        return
    orig = bacc_mod.get_activation_tables
    keep = "natural_log_exp_and_others"
    mine = {AF.Identity, AF.Relu, AF.Ln, AF.Exp, AF.Copy, AF.Square}

    def patched(arch):
        tabs = orig(arch)
        if keep not in tabs or not mine <= tabs[keep]:
            return tabs
        return {name: (funcs if name == keep else funcs - mine)
                for name, funcs in tabs.items()}

    bacc_mod.get_activation_tables = patched
    bacc_mod._act_tables_patched = True


def _get_nc():
    if "nc" not in _CACHE:
        _patch_act_tables()
        nc = build_nc(BPC)
        nc.finalize()
        _CACHE["nc"] = nc
    return _CACHE["nc"]


def make_in_maps(x, fc1_w, fc1_b, fc2_w, fc2_b, un_emb_w, un_emb_b,
                 un_red_w, un_red_b, pw_x_w, pw_x_b, pw_y_w, pw_y_b, red_w):
    w1t = np.ascontiguousarray(fc1_w.T).astype(np.float16)    # [IN(ki), IN(m)]
    # w1m[p, m, k, j] = w1t[k*128+p, m*128+j]
    w1m = np.ascontiguousarray(
        w1t.reshape(KI, 128, KI, 128).transpose(1, 2, 0, 3))
    wr_col = np.ascontiguousarray(un_red_w.T).reshape(KD, 128).transpose(1, 0)
    shared = {
        "w1m": w1m,
        "w2t": np.ascontiguousarray(fc2_w.T).astype(np.float16),
        "wet": np.ascontiguousarray(un_emb_w.T).astype(NP8),
        "wxt": np.ascontiguousarray(pw_x_w.T).astype(NP8),
        "wyt": np.ascontiguousarray(pw_y_w.T).astype(NP8),
        "wrt": np.ascontiguousarray(
            np.repeat(wr_col[:, :, None], 32, axis=2)).astype(NP8),
        "ones8": np.ones([128, 2, 32], NP8),
        "b1": np.asarray(fc1_b, np.float32),
        "b2": np.asarray(fc2_b, np.float32),
        "be": np.asarray(un_emb_b, np.float32),
        "bx": np.asarray(pw_x_b, np.float32),
        "by": np.asarray(pw_y_b, np.float32),
        "consts": np.array([un_red_b[0] * red_w[0], red_w[0], red_w[1], 0.0],
                           np.float32),
        "onesr16": np.ones([1, 128], np.float16),
    }
    in_maps = []
    for c in range(NCORES):
        xs = np.ascontiguousarray(
            x[c * BPC:(c + 1) * BPC].transpose(0, 2, 1)).astype(np.float16)
        in_maps.append({"xt": xs, **shared})
    return in_maps


def kernel(**inputs) -> np.ndarray:
    inputs = {k: np.asarray(v) for k, v in inputs.items()}
    nc = _get_nc()
    in_maps = make_in_maps(**inputs)
    res = run_bass_kernel_spmd(nc, in_maps, core_ids=list(range(NCORES)))
    return np.concatenate([res.results[c]["out"] for c in range(NCORES)], axis=0)


# revision 53
# speedup vs baseline: 1.0604x; 1.0028x over previous
"""Trainium2 Bass kernel for FGAEmbedder (B=32, T=1024, IN=1536, D=768).

Math (identical to the reference up to float reassociation):
    h  = relu(x @ W1^T + b1)           [B,T,IN]
    u  = h @ W2^T + b2                 [B,T,D]
    e  = relu(u @ We^T + be)
    un = e @ Wr^T + br                 [B,T]
    xe = u @ Wx^T + bx ; ye = u @ Wy^T + by
    pw[t] = mean_s cos(xe[t], ye[s]) = (xe[t] . ybar) / ||xe[t]||,
            ybar = mean_s ye[s]/||ye[s]||      (the TxT matrix never exists)
    out = sum_t softmax(rw0*un + rw1*pw)[t] * u[t]

Sharding: data-parallel over batch, 4 batches per core, weights replicated,
no collectives.  Activations are feature-major on chip ([feat, tok]).

Precision strategy: fc1/fc2 run in fp16 (u directly forms the output), but
every matmul that only feeds the softmax scores runs in fp8e4 with DoubleRow
double-pumping (2 contraction rows per cycle): une, un_red, pw_x, pw_y, the
sum-of-squares row reductions, and the q = xe.ybar matmul.  Score errors are
diluted through the softmax; measured end-to-end rel err ~7e-3 vs the 2e-2
gate.  Softmax skips the max subtraction (scores are bounded ~3), which
removes the global-max dependency so per-tile exp/weighted-sum chains can
pipeline.  Inverse norms use exp(-0.5*ln(s)) so every ACT function in the
kernel (Identity/Relu/Ln/Exp) lives in one activation table - no reloads.

Scheduling: the potentials section is ACT/DVE-heavy but PE-light, so all PE
ops that consume its chain results (the 1/||ye|| broadcast, the q matmuls,
the weight broadcasts) are deferred into the NEXT batch's fc1/fc2 m-loops,
where a dense fp16 matmul block hides the chain latency (the PE is
in-order).

DoubleRow notes (discovered on this toolchain):
  - stationary operand must have >=32 active columns (M=32 minimum), so
    single-row reductions use a 32-wide duplicated stationary vector
  - PSUM output tiles must be allocated full [128, N] and sliced; a
    [32, N] PSUM tile silently produces garbage
  - operand layout is [128, 2, N] slices of [128, K, N] tiles (pair on the
    middle dim)
"""

import numpy as np
import ml_dtypes

import concourse.bass as bass
import concourse.bacc as bacc
import concourse.mybir as mybir
import concourse.tile as tile
from concourse.bass_utils import run_bass_kernel_spmd

B, T, IN, D = 32, 1024, 1536, 768
NCORES = 8
BPC = B // NCORES        # batches per core
NT = 512                 # token tile (matmul moving free dim)
NTT = T // NT            # token tiles per batch
KI = IN // 128           # 12 feature tiles of the 1536 dim
KD = D // 128            # 6 feature tiles of the 768 dim
KP = KD // 2             # 3 fp8 DoubleRow pairs of the 768 dim

F8 = mybir.dt.float8e4
F16 = mybir.dt.float16
F32 = mybir.dt.float32
AF = mybir.ActivationFunctionType
ALU = mybir.AluOpType
AX = mybir.AxisListType
DR = mybir.MatmulPerfMode.DoubleRow
NP8 = ml_dtypes.float8_e4m3


def build_nc(bpc: int = BPC) -> bass.Bass:
    nc = bacc.Bacc()

    xt = nc.declare_dram_parameter("xt", [bpc, IN, T], F16, isOutput=False)
    # w1 pre-arranged on host to the exact SBUF layout, m-chunk major:
    # w1m[p, m, k, j] = W1T[k*128+p, m*128+j]
    w1m = nc.declare_dram_parameter("w1m", [128, KI, KI, 128], F16,
                                    isOutput=False)
    w2t = nc.declare_dram_parameter("w2t", [IN, D], F16, isOutput=False)
    wet = nc.declare_dram_parameter("wet", [D, D], F8, isOutput=False)
    wxt = nc.declare_dram_parameter("wxt", [D, D], F8, isOutput=False)
    wyt = nc.declare_dram_parameter("wyt", [D, D], F8, isOutput=False)
    wrt = nc.declare_dram_parameter("wrt", [128, KD, 32], F8, isOutput=False)
    ones8d = nc.declare_dram_parameter("ones8", [128, 2, 32], F8,
                                       isOutput=False)
    b1d = nc.declare_dram_parameter("b1", [IN], F32, isOutput=False)
    b2d = nc.declare_dram_parameter("b2", [D], F32, isOutput=False)
    bed = nc.declare_dram_parameter("be", [D], F32, isOutput=False)
    bxd = nc.declare_dram_parameter("bx", [D], F32, isOutput=False)
    byd = nc.declare_dram_parameter("by", [D], F32, isOutput=False)
    # consts = [un_red_b*red_w0, red_w0, red_w1, 0]
    cst = nc.declare_dram_parameter("consts", [4], F32, isOutput=False)
    onesr16 = nc.declare_dram_parameter("onesr16", [1, 128], F16,
                                        isOutput=False)
    out = nc.declare_dram_parameter("out", [bpc, D], F32, isOutput=True)

    with tile.TileContext(nc) as tc:
        _body(nc, tc, bpc, xt, w1m, w2t, wet, wxt, wyt, wrt, ones8d,
              b1d, b2d, bed, bxd, byd, cst, onesr16, out)
    return nc


def _body(nc, tc, bpc, xt, w1m, w2t, wet, wxt, wyt, wrt, ones8d,
          b1d, b2d, bed, bxd, byd, cst, onesr16, out):
    with (
        tc.tile_pool(name="wpool", bufs=1) as wpool,
        tc.tile_pool(name="u16p", bufs=2) as u16p,
        tc.tile_pool(name="u8p", bufs=2) as u8p,
        tc.tile_pool(name="bat", bufs=1) as bat,
        tc.tile_pool(name="xp", bufs=2) as xp,
        tc.tile_pool(name="hp", bufs=1) as hp,
        tc.tile_pool(name="ep", bufs=1) as ep,
        tc.tile_pool(name="yp", bufs=2) as yp,
        tc.tile_pool(name="sqp", bufs=2) as sqp,
        tc.tile_pool(name="tmpp", bufs=2) as tmpp,
        tc.tile_pool(name="rows", bufs=1) as rows,
        tc.tile_pool(name="rtmp", bufs=2) as rtmp,
        tc.tile_pool(name="bc16p", bufs=2) as bc16p,
        tc.tile_pool(name="mmp", bufs=5, space="PSUM") as mmp,
        tc.tile_pool(name="rpp", bufs=1, space="PSUM") as rpp,
        tc.tile_pool(name="bcp", bufs=1, space="PSUM") as bcp,
        tc.tile_pool(name="stkp", bufs=1, space="PSUM") as stkp,
    ):
        # ---- persistent weights / constants ----
        # DMA order: first half of x(b0,ti0), then w1 m-chunk 0, then the
        # rest - fc1's first m-block starts after ~1.2MB instead of 6MB.
        NH = NT // 2
        first_xt = xp.tile([128, KI, NT], F16, tag="xt")
        x0r = xt[0].rearrange("(ko p) t -> p ko t", p=128)
        nc.sync.dma_start(first_xt[:, :, 0:NH], x0r[:, :, 0:NH])
        w1_sb = wpool.tile([128, KI, KI, 128], F16)
        nc.sync.dma_start(w1_sb[:, 0], w1m[:, 0])
        nc.sync.dma_start(first_xt[:, :, NH:NT], x0r[:, :, NH:NT])
        b1_sb = wpool.tile([128, KI], F32)
        nc.sync.dma_start(b1_sb, b1d.rearrange("(o p) -> p o", p=128))
        for m in range(1, KI):
            nc.sync.dma_start(w1_sb[:, m], w1m[:, m])
        w2_sb = wpool.tile([128, KI, D], F16)
        w2r = w2t.rearrange("(ko p) m -> p ko m", p=128)
        for k in range(KI):
            nc.sync.dma_start(w2_sb[:, k, :], w2r[:, k, :])
        we_sb = wpool.tile([128, KD, D], F8)
        nc.sync.dma_start(we_sb, wet.rearrange("(ko p) m -> p ko m", p=128))
        wx_sb = wpool.tile([128, KD, D], F8)
        nc.sync.dma_start(wx_sb, wxt.rearrange("(ko p) m -> p ko m", p=128))
        wy_sb = wpool.tile([128, KD, D], F8)
        nc.sync.dma_start(wy_sb, wyt.rearrange("(ko p) m -> p ko m", p=128))
        wr_sb = wpool.tile([128, KD, 32], F8)
        nc.sync.dma_start(wr_sb, wrt[:, :, :])
        ones8_sb = wpool.tile([128, 2, 32], F8)
        nc.sync.dma_start(ones8_sb, ones8d[:, :, :])
        b2_sb = wpool.tile([128, KD], F32)
        nc.sync.dma_start(b2_sb, b2d.rearrange("(o p) -> p o", p=128))
        be_sb = wpool.tile([128, KD], F32)
        nc.sync.dma_start(be_sb, bed.rearrange("(o p) -> p o", p=128))
        bx_sb = wpool.tile([128, KD], F32)
        nc.sync.dma_start(bx_sb, bxd.rearrange("(o p) -> p o", p=128))
        by_sb = wpool.tile([128, KD], F32)
        nc.sync.dma_start(by_sb, byd.rearrange("(o p) -> p o", p=128))
        c_sb = wpool.tile([1, 4], F32)
        nc.sync.dma_start(c_sb, cst[None, :])
        onesr16_sb = wpool.tile([1, 128], F16)
        nc.sync.dma_start(onesr16_sb, onesr16[:, :])

        def alloc_batch(b):
            st = {"b": b}
            st["u16"] = u16p.tile([128, KD, T], F16, tag="u16", name=f"u16_{b}")
            st["u8"] = u8p.tile([128, KD, T], F8, tag="u8", name=f"u8_{b}")
            st["xe8"] = bat.tile([128, KD, T], F8, tag="xe8", name=f"xe_{b}")
            st["ybp"] = bat.tile([128, KD, NTT], F32, tag="ybp", name=f"yp_{b}")
            st["invx"] = rows.tile([1, T], F32, tag="invx", name=f"ix_{b}")
            st["scores"] = rows.tile([1, T], F32, tag="scores", name=f"sc_{b}")
            st["ewh"] = rows.tile([1, T], F16, tag="ewh", name=f"ew_{b}")
            st["smp"] = rows.tile([1, NTT], F32, tag="smp", name=f"sp_{b}")
            st["oacc"] = bat.tile([128, KD, NTT], F32, tag="oacc",
                                  name=f"oa_{b}")
            st["ye"] = [None] * NTT
            return st

        def fc1_part(st, ti, interleave=None):
            """interleave: optional fn(m) emitting deferred ops between
            m-blocks (hides the previous batch's reduction tail)."""
            b = st["b"]
            ns = slice(ti * NT, (ti + 1) * NT)
            halves = None
            if b == 0 and ti == 0:
                xt_sb = first_xt
                halves = (first_xt[:, :, 0:NH], first_xt[:, :, NH:NT])
            else:
                xt_sb = xp.tile([128, KI, NT], F16, tag="xt", name=f"xt{b}_{ti}")
                nc.sync.dma_start(
                    xt_sb,
                    xt[b].rearrange("(ko p) t -> p ko t", p=128)[:, :, ns])
            h = hp.tile([128, KI, NT], F16, tag="h", name=f"h{b}_{ti}")
            for m in range(KI):
                ps = mmp.tile([128, NT], F32, tag="mm")
                if halves is not None:
                    # startup: col-halved accumulation groups so compute
                    # starts as soon as the first half-tile of x lands
                    for hx, xh in enumerate(halves):
                        cs = slice(hx * NH, (hx + 1) * NH)
                        for k in range(KI):
                            nc.tensor.matmul(ps[:, cs], w1_sb[:, m, k, :],
                                             xh[:, k, :],
                                             start=(k == 0),
                                             stop=(k == KI - 1))
                else:
                    for k in range(KI):
                        nc.tensor.matmul(ps, w1_sb[:, m, k, :], xt_sb[:, k, :],
                                         start=(k == 0), stop=(k == KI - 1))
                nc.scalar.activation(h[:, m, :], ps, AF.Relu,
                                     bias=b1_sb[:, m:m + 1])
                if interleave is not None:
                    interleave(m)
            return h

        def fc2_part(st, ti, h, interleave=None):
            ns = slice(ti * NT, (ti + 1) * NT)
            for m in range(KD):
                ps = mmp.tile([128, NT], F32, tag="mm")
                for k in range(KI):
                    nc.tensor.matmul(ps, w2_sb[:, k, m * 128:(m + 1) * 128],
                                     h[:, k, :],
                                     start=(k == 0), stop=(k == KI - 1))
                nc.scalar.activation(st["u16"][:, m, ns], ps, AF.Identity,
                                     bias=b2_sb[:, m:m + 1])
                # second ACT read of the same PSUM emits the fp8 copy; keeps
                # the cast off the DVE queue and lets une start per-m
                nc.scalar.activation(st["u8"][:, m, ns], ps, AF.Identity,
                                     bias=b2_sb[:, m:m + 1])
                if interleave is not None:
                    interleave(m)

        def dr_mm(ps_out, w8, u8ap, tile_position=None):
            """3-pair fp8 DoubleRow accumulation over the 768 dim."""
            for j in range(KP):
                nc.tensor.matmul(ps_out, w8[:, 2 * j:2 * j + 2, :],
                                 u8ap[:, 2 * j:2 * j + 2, :],
                                 start=(j == 0), stop=(j == KP - 1),
                                 perf_mode=DR, tile_position=tile_position)

        def une_part(st, ti):
            b = st["b"]
            ns = slice(ti * NT, (ti + 1) * NT)
            e = ep.tile([128, KD, NT], F8, tag="e", name=f"e{b}_{ti}")
            for m in range(KD):
                ps = mmp.tile([128, NT], F32, tag="mm")
                dr_mm(ps, we_sb[:, :, m * 128:(m + 1) * 128],
                      st["u8"][:, :, ns])
                if m in (1, 4):
                    # ACT saturates on PSUM drains in the potentials phases;
                    # DVE takes every third one (relu via add+max)
                    nc.vector.tensor_scalar(e[:, m, :], ps,
                                            be_sb[:, m:m + 1], 0.0,
                                            ALU.add, ALU.max)
                else:
                    nc.scalar.activation(e[:, m, :], ps, AF.Relu,
                                         bias=be_sb[:, m:m + 1])
            st["e_%d" % ti] = e

        def unred_part(st, ti):
            # scores[ns] = rw0*un_pot = rw0*(Wr e) + rw0*br   (consts folded)
            ns = slice(ti * NT, (ti + 1) * NT)
            rps = rpp.tile([128, NT], F32, tag="row")
            dr_mm(rps[0:32, :], wr_sb, st["e_%d" % ti][:, :, :])
            nc.scalar.activation(st["scores"][:, ns], rps[0:1, :], AF.Identity,
                                 bias=c_sb[:, 0:1], scale=c_sb[:, 1:2])

        def pwx_part(st, ti):
            b = st["b"]
            pool_sq = b < bpc - 1   # exposed tail: keep sq off the slow Pool
            ns = slice(ti * NT, (ti + 1) * NT)
            xe8 = st["xe8"]
            sq = sqp.tile([128, KD, NT], F8, tag="sq", name=f"sx{b}_{ti}")
            for m in range(KD):
                ps = mmp.tile([128, NT], F32, tag="mm")
                dr_mm(ps, wx_sb[:, :, m * 128:(m + 1) * 128],
                      st["u8"][:, :, ns])
                if m in (1, 4):
                    nc.vector.tensor_scalar_add(xe8[:, m, ns], ps,
                                                bx_sb[:, m:m + 1])
                else:
                    nc.scalar.activation(xe8[:, m, ns], ps, AF.Identity,
                                         bias=bx_sb[:, m:m + 1])
                if m % 3 == 2:
                    hs = slice(m - 2, m + 1)
                    eng = nc.vector if (m == 2 or not pool_sq) else nc.gpsimd
                    eng.tensor_mul(sq[:, hs, :], xe8[:, hs, ns],
                                   xe8[:, hs, ns])
            st["sqx"] = sq

        def pwy_part(st, ti):
            b = st["b"]
            pool_sq = b < bpc - 1
            ns = slice(ti * NT, (ti + 1) * NT)
            ye = yp.tile([128, KD, NT], F16, tag="ye", name=f"ye{b}_{ti}")
            sq = sqp.tile([128, KD, NT], F8, tag="sq", name=f"sy{b}_{ti}")
            for m in range(KD):
                ps = mmp.tile([128, NT], F32, tag="mm")
                dr_mm(ps, wy_sb[:, :, m * 128:(m + 1) * 128],
                      st["u8"][:, :, ns])
                if m in (1, 4):
                    nc.vector.tensor_scalar_add(ye[:, m, :], ps,
                                                by_sb[:, m:m + 1])
                else:
                    nc.scalar.activation(ye[:, m, :], ps, AF.Identity,
                                         bias=by_sb[:, m:m + 1])
                if m % 3 == 2:
                    hs = slice(m - 2, m + 1)
                    eng = nc.vector if (m == 2 or not pool_sq) else nc.gpsimd
                    eng.tensor_mul(sq[:, hs, :], ye[:, hs, :],
                                   ye[:, hs, :])
            st["ye"][ti] = ye
            st["sqy"] = sq

        def sumsq_part(st, ti):
            # ssy and ssx in separate PSUM banks, both at partition base 0
            # (DoubleRow dst must start at partition 0 on this toolchain).
            # 1/sqrt via exp(-0.5*ln(s)): stays in the one loaded ACT table.
            ns = slice(ti * NT, (ti + 1) * NT)
            stk = stkp.tile([128, NT], F32, tag="stk")
            for j in range(KP):
                nc.tensor.matmul(stk[0:32, :], ones8_sb,
                                 st["sqy"][:, 2 * j:2 * j + 2, :],
                                 start=(j == 0), stop=(j == KP - 1),
                                 perf_mode=DR)
            rpx = rpp.tile([128, NT], F32, tag="row")
            for j in range(KP):
                nc.tensor.matmul(rpx[0:32, :], ones8_sb,
                                 st["sqx"][:, 2 * j:2 * j + 2, :],
                                 start=(j == 0), stop=(j == KP - 1),
                                 perf_mode=DR)
            t1 = rtmp.tile([1, NT], F32, tag="rt")
            nc.scalar.activation(t1, stk[0:1, :], AF.Ln)
            th = rtmp.tile([1, NT], F16, tag="rth")
            nc.scalar.activation(th, t1, AF.Exp, scale=-0.5)
            t0 = rtmp.tile([1, NT], F32, tag="rt")
            nc.scalar.activation(t0, rpx[0:1, :], AF.Ln)
            t0h = rtmp.tile([1, NT], F16, tag="sxr")
            nc.scalar.activation(t0h, t0, AF.Exp, scale=-0.5)
            # pre-scale by red_w1 so pass2 skips that op
            nc.vector.tensor_scalar_mul(st["invx"][:, ns], t0h, c_sb[:, 2:3])
            st["t1h_%d" % ti] = th

        def split_reduce(dst, tmp, tail):
            """dst[:, :, 0:1] = sum_X tmp.  In the exposed tail, split the
            reduction between DVE and ACT (accum_out) so it drains ~2x
            faster; in the pipeline one DVE op is cheaper."""
            if not tail:
                nc.vector.reduce_sum(dst, tmp, axis=AX.X)
                return
            nc.vector.reduce_sum(dst[:, 0:3], tmp[:, 0:3, :], axis=AX.X)
            for m in range(3, KD):
                nc.scalar.activation(tmp[:, m, :], tmp[:, m, :], AF.Identity,
                                     accum_out=dst[:, m, 0:1])

        def y_pe(st, ti, tail=False):
            # broadcast 1/||ye|| and reduce yn into ybar parts
            ye = st["ye"][ti]
            ivb = bcp.tile([128, NT], F32, tag="bc")
            nc.tensor.matmul(ivb, onesr16_sb, st["t1h_%d" % ti],
                             start=True, stop=True)
            ivb16 = bc16p.tile([128, 1, NT], F16, tag="bc16")
            nc.scalar.activation(ivb16[:, 0, :], ivb, AF.Identity)
            tmp = tmpp.tile([128, KD, NT], F16, tag="tmp")
            nc.vector.tensor_mul(tmp, ye,
                                 ivb16.broadcast_to([128, KD, NT]))
            split_reduce(st["ybp"][:, :, ti:ti + 1], tmp, tail)
            if ti == NTT - 1:
                b = st["b"]
                # 32-wide duplicated fp8 stationary for the q matmul
                ybar8 = bat.tile([128, KD, 32], F8, tag="ybar", name=f"yb{b}")
                ybf = bat.tile([128, KD, 1], F32, tag="ybf", name=f"yf{b}")
                nc.vector.tensor_add(ybf, st["ybp"][:, :, 0:1],
                                     st["ybp"][:, :, 1:2])
                nc.vector.tensor_scalar_mul(ybf, ybf, 1.0 / T)
                nc.vector.tensor_copy(ybar8, ybf.broadcast_to([128, KD, 32]))
                st["ybar8"] = ybar8

        def pass2_q(st, ti):
            # q = xe . ybar ; scores[ns] += rw1 * q * invx ; exp -> ewh
            ns = slice(ti * NT, (ti + 1) * NT)
            qps = rpp.tile([128, NT], F32, tag="row")
            dr_mm(qps[0:32, :], st["ybar8"], st["xe8"][:, :, ns])
            s0 = rtmp.tile([1, NT], F32, tag="rt")
            nc.vector.tensor_mul(s0, qps[0:1, :], st["invx"][:, ns])
            nc.vector.tensor_add(s0, s0, st["scores"][:, ns])
            # no max subtraction: scores are bounded (~3), exp is safe
            nc.scalar.activation(st["ewh"][:, ns], s0, AF.Exp)
            nc.vector.reduce_sum(st["smp"][:, ti:ti + 1], st["ewh"][:, ns],
                                 axis=AX.X)

        def pass2_w(st, ti, tail=False):
            # oacc[:, :, ti] = sum_{t in ns} ewh[t] * u16[:, t]
            ns = slice(ti * NT, (ti + 1) * NT)
            wbc = bcp.tile([128, NT], F32, tag="bc")
            nc.tensor.matmul(wbc, onesr16_sb, st["ewh"][:, ns],
                             start=True, stop=True)
            wbc16 = bc16p.tile([128, 1, NT], F16, tag="bc16")
            nc.scalar.activation(wbc16[:, 0, :], wbc, AF.Identity)
            tmp = tmpp.tile([128, KD, NT], F16, tag="tmp")
            nc.vector.tensor_mul(tmp, st["u16"][:, :, ns],
                                 wbc16.broadcast_to([128, KD, NT]))
            split_reduce(st["oacc"][:, :, ti:ti + 1], tmp, tail)

        def pass2_fin(st):
            b = st["b"]
            sm = rows.tile([1, 1], F32, tag="sm", name=f"sm{b}")
            nc.vector.tensor_add(sm, st["smp"][:, 0:1], st["smp"][:, 1:2])
            nc.vector.reciprocal(sm, sm)
            smh = rows.tile([1, 1], F16, tag="smh", name=f"sh{b}")
            nc.vector.tensor_copy(smh, sm)
            smb = bcp.tile([128, 1], F32, tag="bc")
            nc.tensor.matmul(smb, onesr16_sb, smh, start=True, stop=True)
            ofin = bat.tile([128, KD, 1], F32, tag="ofin", name=f"of{b}")
            nc.vector.tensor_add(ofin, st["oacc"][:, :, 0:1],
                                 st["oacc"][:, :, 1:2])
            nc.vector.tensor_scalar_mul(ofin, ofin, smb)
            nc.sync.dma_start(out[b].rearrange("(mo p) -> p mo", p=128),
                              ofin[:, :, 0])

        def il_fc1(prev):
            """prev's reduction tail, spread across fc1's 12 m-blocks."""
            if prev is None:
                return None

            def f(m):
                if m == 1:
                    y_pe(prev, 0)
                elif m == 3:
                    y_pe(prev, 1)     # includes the ybar8 chain
                elif m == 6:
                    pass2_q(prev, 0)
                elif m == 8:
                    pass2_q(prev, 1)
                elif m == 10:
                    pass2_w(prev, 0)
            return f

        def il_fc2(prev):
            if prev is None:
                return None

            def f(m):
                if m == 1:
                    pass2_w(prev, 1)
                elif m == 3:
                    pass2_fin(prev)
            return f

        prev = None
        for b in range(bpc):
            st = alloc_batch(b)
            h0 = fc1_part(st, 0, interleave=il_fc1(prev))
            fc2_part(st, 0, h0, interleave=il_fc2(prev))
            h1 = fc1_part(st, 1)
            fc2_part(st, 1, h1)
            une_part(st, 0)
            pwx_part(st, 0)
            pwy_part(st, 0)
            unred_part(st, 0)
            sumsq_part(st, 0)
            une_part(st, 1)
            if b == bpc - 1:
                # last batch: ti0's ybar contribution runs under ti1's
                # potentials so only ti1's short chain is exposed in the tail
                y_pe(st, 0, tail=True)
            pwx_part(st, 1)
            pwy_part(st, 1)
            unred_part(st, 1)
            sumsq_part(st, 1)
            prev = st
        y_pe(prev, 1, tail=True)
        pass2_q(prev, 0)
        pass2_q(prev, 1)
        pass2_w(prev, 0, tail=True)
        pass2_w(prev, 1, tail=True)
        pass2_fin(prev)


_CACHE = {}


def _patch_act_tables():
    """Steer every activation to the one table that contains all the funcs
    this kernel uses (Identity/Relu/Ln/Exp), so the ACT engine loads its
    piecewise-polynomial table once instead of ping-ponging between the
    exp and ln tables every batch (1.3us per reload, on the critical path).
    Table order (= act_func_set_id) is preserved; only the func sets the
    placement pass sees are filtered."""
    import concourse.bacc as bacc_mod
    if getattr(bacc_mod, "_act_tables_patched", False):
        return
    orig = bacc_mod.get_activation_tables
    keep = "natural_log_exp_and_others"
    mine = {AF.Identity, AF.Relu, AF.Ln, AF.Exp, AF.Copy, AF.Square}

    def patched(arch):
        tabs = orig(arch)
        if keep not in tabs or not mine <= tabs[keep]:
            return tabs
        return {name: (funcs if name == keep else funcs - mine)
                for name, funcs in tabs.items()}

    bacc_mod.get_activation_tables = patched
    bacc_mod._act_tables_patched = True


def _get_nc():
    if "nc" not in _CACHE:
        _patch_act_tables()
        nc = build_nc(BPC)
        nc.finalize()
        _CACHE["nc"] = nc
    return _CACHE["nc"]


def make_in_maps(x, fc1_w, fc1_b, fc2_w, fc2_b, un_emb_w, un_emb_b,
                 un_red_w, un_red_b, pw_x_w, pw_x_b, pw_y_w, pw_y_b, red_w):
    w1t = np.ascontiguousarray(fc1_w.T).astype(np.float16)    # [IN(ki), IN(m)]
    # w1m[p, m, k, j] = w1t[k*128+p, m*128+j]
    w1m = np.ascontiguousarray(
        w1t.reshape(KI, 128, KI, 128).transpose(1, 2, 0, 3))
    wr_col = np.ascontiguousarray(un_red_w.T).reshape(KD, 128).transpose(1, 0)
    shared = {
        "w1m": w1m,
        "w2t": np.ascontiguousarray(fc2_w.T).astype(np.float16),
        "wet": np.ascontiguousarray(un_emb_w.T).astype(NP8),
        "wxt": np.ascontiguousarray(pw_x_w.T).astype(NP8),
        "wyt": np.ascontiguousarray(pw_y_w.T).astype(NP8),
        "wrt": np.ascontiguousarray(
            np.repeat(wr_col[:, :, None], 32, axis=2)).astype(NP8),
        "ones8": np.ones([128, 2, 32], NP8),
        "b1": np.asarray(fc1_b, np.float32),
        "b2": np.asarray(fc2_b, np.float32),
        "be": np.asarray(un_emb_b, np.float32),
        "bx": np.asarray(pw_x_b, np.float32),
        "by": np.asarray(pw_y_b, np.float32),
        "consts": np.array([un_red_b[0] * red_w[0], red_w[0], red_w[1], 0.0],
                           np.float32),
        "onesr16": np.ones([1, 128], np.float16),
    }
    in_maps = []
    for c in range(NCORES):
        xs = np.ascontiguousarray(
            x[c * BPC:(c + 1) * BPC].transpose(0, 2, 1)).astype(np.float16)
        in_maps.append({"xt": xs, **shared})
    return in_maps


def kernel(**inputs) -> np.ndarray:
    inputs = {k: np.asarray(v) for k, v in inputs.items()}
    nc = _get_nc()
    in_maps = make_in_maps(**inputs)
    res = run_bass_kernel_spmd(nc, in_maps, core_ids=list(range(NCORES)))
    return np.concatenate([res.results[c]["out"] for c in range(NCORES)], axis=0)
